# revision 52
# baseline (speedup 1.0000x reference)
"""Trainium2 Bass kernel for nn_NavigationNet.

Math notes (validated host-side vs the jax reference, rel-l2 ~3e-6):

- `teom` is never used by the network (the conv path is dead code): the
  teom-LSTM consumes a hard-zero input with zero initial state, so its
  hidden-state trajectory is identical for every batch element.  Its entire
  contribution reduces to a precomputable per-step bias vector, which we fold
  through Wtf/Wmix/W1 into a [32, 64] table `BC` feeding the first MLP layer.
- `c_out` is exactly zeros.
- The mix head `W1 @ (Wmix[:,64:] @ (Wof @ h))` collapses into one [32,64]
  matrix `Wc`.
- All four LSTM gates use a single sigmoid activation per step: the g-gate
  pre-activation rows are pre-scaled by 2 so tanh(g) = 2*sigmoid(2g) - 1,
  and the cell state is tracked as c/2 so the trailing *2 folds into the
  Tanh activation's free `scale` operand.
- The y feedback `y_t = o_t + 1.5*y_{t-1} - 0.5*y_{t-3}` is kept in a
  [2, 67*128] ring (3 seed slots hold obsv[:, 5:8]).

Sharding: pure data parallel, batch 1024 -> 8 cores x 128.  Per core the
layout is feature-on-partition, batch-on-free (128 batch lanes in the free
dimension), so every op in the recurrence is a small [P<=128, 128] tile op.
"""

import numpy as np

import concourse.bass as bass
import concourse.mybir as mybir
import concourse.tile as tile
from concourse.bass_utils import run_bass_kernel_spmd
from concourse.tile import add_dep_helper
from concourse.vector_clock import ScopedClock

F32 = mybir.dt.float32
BF16 = mybir.dt.bfloat16
AF = mybir.ActivationFunctionType
ALU = mybir.AluOpType

BS, OBS_LEN, TS, HID = 1024, 8, 64, 64
N_CORES = 8
BSH = BS // N_CORES  # 128 batch per core


# ---------------------------------------------------------------------------
# walrus in this container rejects >1 sync-wait per instruction; split them.
class _TileContextSW(tile.TileContext):
    def _drain_and_barrier(self, tick_clock, wait_clock):
        probe = self.nc.sync.nop(hint="wait_probe", nofuse=True)
        wait_clock.add_sem_waits(
            probe.ins, ScopedClock({None: tick_clock.global_clock})
        )
        si = probe.ins.sync_info
        waits = list(si.on_wait) if si is not None and si.on_wait else []
        probe.ins.sync_info = None
        for w in waits:
            n = self.nc.sync.nop(hint="split_wait", nofuse=True)
            n.ins.sync_info = mybir.SyncInfo(on_wait=[w], on_update=[])
        self.nc.sync.drain()
        self.nc.all_engine_barrier()
        assert self.sems is not None
        popped = self.nc._tile_sem_poison_stack.pop()
        assert popped is self._sem_poison
        self.nc.clear_and_free_semaphores(list(self.sems.allocated().values()))
        self.nc.all_engine_barrier()


def _split_multi_waits(nc):
    for fn in nc.m.functions:
        for blk in fn.blocks:
            insts = blk.instructions
            out = []
            changed = False
            for inst in insts:
                si = inst.sync_info
                waits = list(si.on_wait) if si is not None and si.on_wait else []
                if len(waits) > 1:
                    for j, w in enumerate(waits[:-1]):
                        out.append(
                            mybir.InstNoOp(
                                name=f"{inst.name}-sw{j}",
                                engine=inst.engine,
                                ins=[],
                                outs=[],
                                sync_info=mybir.SyncInfo(on_wait=[w], on_update=[]),
                                bass_nofuse=True,
                            )
                        )
                    inst.sync_info = mybir.SyncInfo(
                        on_wait=[waits[-1]],
                        on_update=list(si.on_update) if si.on_update else [],
                    )
                    changed = True
                out.append(inst)
            if changed:
                insts[:] = out


# ---------------------------------------------------------------------------
# Host-side constant folding (float32 throughout, like the reference).
def _host_tables(W):
    f32 = np.float32
    sig = lambda v: (1.0 / (1.0 + np.exp(-v, dtype=np.float64))).astype(f32)
    tanh = lambda v: np.tanh(v.astype(np.float64)).astype(f32)

    b_o = W["bih_o"] + W["bhh_o"]
    b_t = W["bih_t"] + W["bhh_t"]
    Wmix_t = W["Wmix"][:, :64]
    Wmix_o = W["Wmix"][:, 64:]
    base = Wmix_o @ W["bof"] + W["bmix"]

    h = np.zeros(HID, f32)
    c = np.zeros(HID, f32)
    BC = np.zeros((32, TS), f32)
    for t in range(TS):
        g = W["Whh_t"] @ h + b_t
        i, f, gg, o = g[:64], g[64:128], g[128:192], g[192:256]
        c = sig(f) * c + sig(i) * tanh(gg)
        h = sig(o) * tanh(c)
        ts_ = W["Wtf"] @ h + W["btf"]
        BC[:, t] = W["W1"] @ (Wmix_t @ ts_ + base) + W["b1"]
    Wc = W["W1"] @ Wmix_o @ W["Wof"]

    Wh = W["Whh_o"].copy()  # [256, 64]
    Wx = W["Wih_o"].copy()  # [256, 2]
    bb = b_o.copy()
    Wh[128:192] *= 2.0  # g-gate rows via tanh(x) = 2 sig(2x) - 1
    Wx[128:192] *= 2.0
    bb[128:192] *= 2.0
    # gate-row order [f; i; o; g2]: keeps every later tensor-tensor pair on
    # matching base partitions (f/o at 0, i/g at 64).
    perm = np.concatenate([
        np.arange(64, 128), np.arange(0, 64),
        np.arange(192, 256), np.arange(128, 192)])
    Wh, Wx, bb = Wh[perm], Wx[perm], bb[perm]
    WxW3 = Wx @ W["W3"]  # [256, 32]
    # rhs supertile Z rows: [h 0:64 | a2 64:96 | w 96:98 | ones 98]
    # (the scan reuses rows 64:66 for x with its own ones row at 66)
    Wmain = np.concatenate([Wh, WxW3, Wx, bb[:, None]], axis=1)  # [256, 99]
    Wscan = np.concatenate([Wh, Wx, bb[:, None]], axis=1)        # [256, 67]

    import ml_dtypes
    bf16 = ml_dtypes.bfloat16
    zinit = np.zeros((99, BSH), np.float32)
    zinit[66, :] = 1.0
    zinit[98, :] = 1.0
    # lhsT_w3 padded so its slice starts at base partition 64 to match the
    # a2 rows of Z (matmul requires equal base partitions).
    w3_p = np.zeros((96, 2), np.float32)
    w3_p[64:96] = W["W3"].T
    return dict(
        lhsT_main=np.ascontiguousarray(Wmain.T.astype(bf16)),  # [99, 256]
        lhsT_scan=np.ascontiguousarray(Wscan.T.astype(bf16)),  # [67, 256]
        lhsT_wc=np.ascontiguousarray(Wc.T.astype(bf16)),       # [64, 32]
        lhsT_w2=np.ascontiguousarray(W["W2"].T.astype(bf16)),  # [32, 32]
        lhsT_w3=np.ascontiguousarray(w3_p.astype(bf16)),       # [96, 2]
        BCT=BC,                                     # [32, 64]
        b2c=np.ascontiguousarray(W["b2"][:, None]),  # [32, 1]
        b3=W["b3"],                                  # [2]
        ident2=np.eye(2, dtype=f32),                 # [2, 2]
        zinit=np.ascontiguousarray(zinit.astype(bf16)),  # [67, BSH]
    )


# ---------------------------------------------------------------------------
def _build_bass():
    nc = bass.Bass()

    din = {}
    for name, shape, dt in [
        ("obsT", [2, OBS_LEN * BSH], BF16),
        ("ys0", [2, 3 * BSH], F32),
        ("w0", [2, BSH], F32),
        ("b3b", [2, BSH], F32),
        ("lhsT_main", [99, 256], BF16),
        ("lhsT_scan", [67, 256], BF16),
        ("lhsT_wc", [64, 32], BF16),
        ("lhsT_w2", [32, 32], BF16),
        ("lhsT_w3", [96, 2], BF16),
        ("BCT", [32, TS], F32),
        ("b2c", [32, 1], F32),
        ("ident2", [2, 2], F32),
        ("zinit", [99, BSH], BF16),
    ]:
        din[name] = nc.dram_tensor(name, shape, dt, kind="ExternalInput")
    y_out = nc.dram_tensor("y", [BSH, 2 * TS], F32, kind="ExternalOutput")

    with _TileContextSW(nc) as tc:
        with (
            tc.tile_pool(name="consts", bufs=1) as cp,
            tc.tile_pool(name="work", bufs=2) as wk,
            tc.tile_pool(name="pg", bufs=2, space="PSUM") as pg,
            tc.tile_pool(name="pmix", bufs=2, space="PSUM") as pmix,
            tc.tile_pool(name="pyp", bufs=1, space="PSUM") as pyp,
            tc.tile_pool(name="pout", bufs=1, space="PSUM") as pout,
        ):
            # --- constants into SBUF
            sb = {}
            for name, t in din.items():
                sb[name] = cp.tile(list(t.shape), t.dtype, tag=name, name=name)
                nc.sync.dma_start(out=sb[name], in_=t[:, :])

            # Z: the gate-matmul rhs supertile.
            # rows [h 0:64 | a2 64:96 | w 96:98 | ones 98]  (all bf16);
            # the scan uses rows 64:66 for x with its own ones row at 66.
            Z = cp.tile([99, BSH], BF16, tag="Z")
            ch = cp.tile([64, BSH], F32, tag="ch")     # c/2
            ys = cp.tile([2, (TS + 3) * BSH], F32, tag="ys")
            O_sb = cp.tile([BSH, 2 * TS], F32, tag="O_sb")

            nc.sync.dma_start(out=Z[:, :], in_=din["zinit"][:, :])
            nc.vector.memset(ch, 0.0)
            nc.sync.dma_start(out=ys[:, 0 : 3 * BSH], in_=din["ys0"][:, :])

            O_ps = pout.tile([BSH, 2 * TS], F32, tag="O_ps")

            c15 = cp.tile([2, BSH], F32, tag="c15")
            cm05 = cp.tile([2, BSH], F32, tag="cm05")
            nc.vector.memset(c15, 1.5)
            nc.vector.memset(cm05, -0.5)

            lmain, lscan = sb["lhsT_main"], sb["lhsT_scan"]
            obsT = sb["obsT"]
            Zh = Z[0:64, :]      # h rows
            Zxs = Z[64:66, :]    # x rows (scan)
            Za2 = Z[64:96, :]    # a2 rows (main)
            Zw = Z[96:98, :]     # w rows (main)

            pend = {"cast": None}

            def state_update(G):
                """sigmoid(G) -> new ch (c/2) and h (Z rows 0:64, bf16)."""
                S = wk.tile([128, 256], F32, tag="S")
                nc.scalar.activation(out=S, in_=G, func=AF.Sigmoid)
                S_f, S_i = S[0:64, 0:128], S[64:128, 0:128]
                S_o, S_g = S[0:64, 128:256], S[64:128, 128:256]
                tt = wk.tile([64, BSH], F32, tag="tt")
                tt_i = nc.vector.scalar_tensor_tensor(
                    out=tt, in0=S_g, scalar=0.5, in1=S_i,
                    op0=ALU.subtract, op1=ALU.mult)
                if pend["cast"] is not None:
                    # keep the off-path Zw cast out of the on-path VE window
                    add_dep_helper(tt_i.ins, pend["cast"].ins, sync=False,
                                   reason="Zw cast before next state update")
                    pend["cast"] = None
                m2 = wk.tile([64, BSH], F32, tag="m2")
                nc.vector.tensor_mul(out=m2, in0=S_f, in1=ch)
                nc.vector.tensor_add(out=ch, in0=tt, in1=m2)
                th = wk.tile([64, BSH], F32, tag="th")
                nc.scalar.activation(out=th, in_=ch, func=AF.Tanh, scale=2.0)
                nc.vector.tensor_mul(out=Zh, in0=S_o, in1=th)

            # --- warmup scan over the observed trajectory (K=67 gate matmuls)
            G_cur = None
            for k in range(OBS_LEN):
                nc.vector.tensor_copy(Zxs, obsT[:, k * BSH : (k + 1) * BSH])
                G = pg.tile([128, 256], F32, tag="G")
                nc.tensor.matmul(G[:, 0:128], lscan[:, 0:128], Z[0:67, :],
                                 start=True, stop=True)
                nc.tensor.matmul(G[:, 128:256], lscan[:, 128:256],
                                 Z[0:67, :], start=True, stop=True)
                state_update(G)

            # --- 64 output steps (software-pipelined)
            # Gates for step t+1 are ONE matmul per half with rhs Z:
            #   G(t+1) = [WxW3 | Wh | Wx | b] . [a2_t; h_t; w_t; 1]
            # using y_t = W3.a2_t + w_t, so Wx.y_t = WxW3.a2_t + Wx.w_t and the
            # only on-path dependency is a2_t (w_t is known since step t-1).
            w_cur = cp.tile([2, BSH], F32, tag="w0_t")
            nc.sync.dma_start(out=w_cur, in_=din["w0"][:, :])
            nc.vector.tensor_copy(Zw, w_cur)  # bf16 w for the gate matmul

            for t in range(TS):
                # -0.5*y_{t-2} + b3, ready since step t-2 (Pool, off-path)
                vtmp = wk.tile([2, BSH], F32, tag="vtmp")
                nc.gpsimd.tensor_mul(out=vtmp, in0=cm05,
                                     in1=ys[:, (t + 1) * BSH : (t + 2) * BSH])
                nc.gpsimd.tensor_add(out=vtmp, in0=vtmp, in1=sb["b3b"])
                if t > 0:
                    state_update(G_cur)
                build_next = t < TS - 1
                P1 = pmix.tile([32, BSH], F32, tag="P1")
                nc.tensor.matmul(P1, sb["lhsT_wc"], Zh,
                                 start=True, stop=True)
                a1 = wk.tile([32, BSH], BF16, tag="a1")
                nc.scalar.activation(out=a1, in_=P1, func=AF.Prelu,
                                     bias=sb["BCT"][:, t : t + 1], alpha=0.1)
                P2 = pmix.tile([32, BSH], F32, tag="P2")
                nc.tensor.matmul(P2, sb["lhsT_w2"], a1, start=True, stop=True)
                nc.scalar.activation(out=Za2, in_=P2, func=AF.Prelu,
                                     bias=sb["b2c"], alpha=0.1)
                if build_next:
                    G_next = pg.tile([128, 256], F32, tag="G")
                    nc.tensor.matmul(G_next[:, 0:128], lmain[:, 0:128], Z,
                                     start=True, stop=True)
                    nc.tensor.matmul(G_next[:, 128:256], lmain[:, 128:256], Z,
                                     start=True, stop=True)
                    G_cur = G_next
                PY = pyp.tile([2, BSH], F32, tag="PY")
                nc.tensor.matmul(PY, sb["lhsT_w3"][64:96, :], Za2,
                                 start=True, stop=True)
                ys_s = ys[:, (t + 3) * BSH : (t + 4) * BSH]
                yadd_i = nc.vector.tensor_add(out=ys_s, in0=PY, in1=w_cur)
                # accumulate transposed output column pair (off critical path)
                nc.tensor.matmul(O_ps[:, 2 * t : 2 * t + 2], ys_s,
                                 sb["ident2"], is_transpose=True,
                                 start=True, stop=True)
                # w for step t+1 = 1.5*y_t + vtmp  (Pool, off critical path)
                u = wk.tile([2, BSH], F32, tag="u")
                nc.gpsimd.tensor_mul(out=u, in0=c15, in1=ys_s)
                wn = wk.tile([2, BSH], F32, tag="wn")
                nc.gpsimd.tensor_add(out=wn, in0=u, in1=vtmp)
                if build_next:
                    cast_i = nc.vector.tensor_copy(Zw, wn)
                    add_dep_helper(cast_i.ins, yadd_i.ins, sync=False,
                                   reason="Zw cast after y-add on VE")
                    pend["cast"] = cast_i
                w_cur = wn

            nc.scalar.copy(out=O_sb, in_=O_ps)
            nc.sync.dma_start(out=y_out[:, :], in_=O_sb)

    _split_multi_waits(nc)
    return nc


_CACHED = {}


def _get_nc():
    if "nc" not in _CACHED:
        _CACHED["nc"] = _build_bass()
    return _CACHED["nc"]


def _in_maps(inputs):
    f32 = np.float32
    W = {k: np.ascontiguousarray(np.asarray(v), dtype=f32) for k, v in inputs.items()
         if k != "teom"}
    tabs = _host_tables(W)
    b3 = tabs.pop("b3")
    obsv = W["obsv"]  # [1024, 8, 2]

    const_part = {k: np.ascontiguousarray(v) for k, v in tabs.items()}
    const_part["b3b"] = np.ascontiguousarray(
        np.broadcast_to(b3[:, None], (2, BSH)), dtype=f32)

    maps = []
    for i in range(N_CORES):
        sh = obsv[i * BSH : (i + 1) * BSH]  # [128, 8, 2]
        m = dict(const_part)
        import ml_dtypes
        m["obsT"] = np.ascontiguousarray(
            sh.transpose(2, 1, 0).reshape(2, OBS_LEN * BSH).astype(ml_dtypes.bfloat16))
        m["ys0"] = np.ascontiguousarray(
            sh[:, 5:8, :].transpose(2, 1, 0).reshape(2, 3 * BSH))
        w0 = 1.5 * sh[:, 7, :] - 0.5 * sh[:, 5, :] + b3  # [128, 2]
        m["w0"] = np.ascontiguousarray(w0.T)
        maps.append(m)
    return maps


def run(inputs, trace=False, **kw):
    nc = _get_nc()
    res = run_bass_kernel_spmd(
        nc, _in_maps(inputs), core_ids=list(range(N_CORES)), trace=trace, **kw)
    ys = [r["y"].reshape(BSH, TS, 2) for r in res.results]
    y = np.concatenate(ys, axis=0)
    c = np.zeros((BS, TS), np.float32)
    return (y, c), res


def kernel(**inputs):
    (y, c), _ = run(inputs)
    return y, c


# revision 55
# speedup vs baseline: 1.1540x; 1.1540x over previous
"""Trainium2 Bass kernel for nn_NavigationNet.

Math notes (validated host-side vs the jax reference, rel-l2 ~3e-6):

- `teom` is never used by the network (the conv path is dead code): the
  teom-LSTM consumes a hard-zero input with zero initial state, so its
  hidden-state trajectory is identical for every batch element.  Its entire
  contribution reduces to a precomputable per-step bias vector, which we fold
  through Wtf/Wmix/W1 into a [32, 64] table `BC` feeding the first MLP layer.
- `c_out` is exactly zeros.
- The mix head `W1 @ (Wmix[:,64:] @ (Wof @ h))` collapses into one [32,64]
  matrix `Wc`.
- All four LSTM gates use a single sigmoid activation per step: the g-gate
  pre-activation rows are pre-scaled by 2 so tanh(g) = 2*sigmoid(2g) - 1,
  and the cell state is tracked as c/2 so the trailing *2 folds into the
  Tanh activation's free `scale` operand.
- The y feedback `y_t = o_t + 1.5*y_{t-1} - 0.5*y_{t-3}` is kept in a
  [2, 67*128] ring (3 seed slots hold obsv[:, 5:8]).

Sharding: pure data parallel, batch 1024 -> 8 cores x 128.  Per core the
layout is feature-on-partition, batch-on-free (128 batch lanes in the free
dimension), so every op in the recurrence is a small [P<=128, 128] tile op.
"""

import numpy as np

import concourse.bass as bass
import concourse.mybir as mybir
import concourse.tile as tile
from concourse.bass_utils import run_bass_kernel_spmd
from concourse.tile import add_dep_helper
from concourse.vector_clock import ScopedClock

F32 = mybir.dt.float32
BF16 = mybir.dt.bfloat16
AF = mybir.ActivationFunctionType
ALU = mybir.AluOpType

BS, OBS_LEN, TS, HID = 1024, 8, 64, 64
N_CORES = 8
BSH = BS // N_CORES  # 128 batch per core


# ---------------------------------------------------------------------------
# walrus in this container rejects >1 sync-wait per instruction; split them.
class _TileContextSW(tile.TileContext):
    def _drain_and_barrier(self, tick_clock, wait_clock):
        probe = self.nc.sync.nop(hint="wait_probe", nofuse=True)
        wait_clock.add_sem_waits(
            probe.ins, ScopedClock({None: tick_clock.global_clock})
        )
        si = probe.ins.sync_info
        waits = list(si.on_wait) if si is not None and si.on_wait else []
        probe.ins.sync_info = None
        for w in waits:
            n = self.nc.sync.nop(hint="split_wait", nofuse=True)
            n.ins.sync_info = mybir.SyncInfo(on_wait=[w], on_update=[])
        self.nc.sync.drain()
        self.nc.all_engine_barrier()
        assert self.sems is not None
        popped = self.nc._tile_sem_poison_stack.pop()
        assert popped is self._sem_poison
        self.nc.clear_and_free_semaphores(list(self.sems.allocated().values()))
        self.nc.all_engine_barrier()


def _split_multi_waits(nc):
    for fn in nc.m.functions:
        for blk in fn.blocks:
            insts = blk.instructions
            out = []
            changed = False
            for inst in insts:
                si = inst.sync_info
                waits = list(si.on_wait) if si is not None and si.on_wait else []
                if len(waits) > 1:
                    for j, w in enumerate(waits[:-1]):
                        out.append(
                            mybir.InstNoOp(
                                name=f"{inst.name}-sw{j}",
                                engine=inst.engine,
                                ins=[],
                                outs=[],
                                sync_info=mybir.SyncInfo(on_wait=[w], on_update=[]),
                                bass_nofuse=True,
                            )
                        )
                    inst.sync_info = mybir.SyncInfo(
                        on_wait=[waits[-1]],
                        on_update=list(si.on_update) if si.on_update else [],
                    )
                    changed = True
                out.append(inst)
            if changed:
                insts[:] = out


# ---------------------------------------------------------------------------
# Host-side constant folding (float32 throughout, like the reference).
def _host_tables(W):
    f32 = np.float32
    sig = lambda v: (1.0 / (1.0 + np.exp(-v, dtype=np.float64))).astype(f32)
    tanh = lambda v: np.tanh(v.astype(np.float64)).astype(f32)

    b_o = W["bih_o"] + W["bhh_o"]
    b_t = W["bih_t"] + W["bhh_t"]
    Wmix_t = W["Wmix"][:, :64]
    Wmix_o = W["Wmix"][:, 64:]
    base = Wmix_o @ W["bof"] + W["bmix"]

    h = np.zeros(HID, f32)
    c = np.zeros(HID, f32)
    BC = np.zeros((32, TS), f32)
    for t in range(TS):
        g = W["Whh_t"] @ h + b_t
        i, f, gg, o = g[:64], g[64:128], g[128:192], g[192:256]
        c = sig(f) * c + sig(i) * tanh(gg)
        h = sig(o) * tanh(c)
        ts_ = W["Wtf"] @ h + W["btf"]
        BC[:, t] = W["W1"] @ (Wmix_t @ ts_ + base) + W["b1"]
    Wc = W["W1"] @ Wmix_o @ W["Wof"]

    Wh = W["Whh_o"].copy()  # [256, 64]
    Wx = W["Wih_o"].copy()  # [256, 2]
    bb = b_o.copy()
    Wh[128:192] *= 2.0  # g-gate rows via tanh(x) = 2 sig(2x) - 1
    Wx[128:192] *= 2.0
    bb[128:192] *= 2.0
    # gate-row order [f; i; o; g2]: keeps every later tensor-tensor pair on
    # matching base partitions (f/o at 0, i/g at 64).
    perm = np.concatenate([
        np.arange(64, 128), np.arange(0, 64),
        np.arange(192, 256), np.arange(128, 192)])
    Wh, Wx, bb = Wh[perm], Wx[perm], bb[perm]
    WxW3 = Wx @ W["W3"]  # [256, 32]
    # rhs supertile Z rows: [h 0:64 | a2 64:96 | w 96:98 | ones 98]
    # (the scan reuses rows 64:66 for x with its own ones row at 66)
    Wmain = np.concatenate([Wh, WxW3, Wx, bb[:, None]], axis=1)  # [256, 99]
    Wscan = np.concatenate([Wh, Wx, bb[:, None]], axis=1)        # [256, 67]

    import ml_dtypes
    bf16 = ml_dtypes.bfloat16
    zinit = np.zeros((99, BSH), np.float32)
    zinit[66, :] = 1.0
    zinit[98, :] = 1.0
    # lhsT_w3 padded so its slice starts at base partition 64 to match the
    # a2 rows of Z (matmul requires equal base partitions).
    w3_p = np.zeros((96, 2), np.float32)
    w3_p[64:96] = W["W3"].T
    return dict(
        lhsT_main=np.ascontiguousarray(Wmain.T.astype(bf16)),  # [99, 256]
        lhsT_scan=np.ascontiguousarray(Wscan.T.astype(bf16)),  # [67, 256]
        lhsT_wc=np.ascontiguousarray(Wc.T.astype(bf16)),       # [64, 32]
        lhsT_w2=np.ascontiguousarray(W["W2"].T.astype(bf16)),  # [32, 32]
        lhsT_w3=np.ascontiguousarray(w3_p.astype(bf16)),       # [96, 2]
        BCT=BC,                                     # [32, 64]
        b2c=np.ascontiguousarray(W["b2"][:, None]),  # [32, 1]
        b3=W["b3"],                                  # [2]
        ident2=np.eye(2, dtype=f32),                 # [2, 2]
        zinit=np.ascontiguousarray(zinit.astype(bf16)),  # [67, BSH]
    )


# ---------------------------------------------------------------------------
def _build_bass():
    nc = bass.Bass()

    din = {}
    for name, shape, dt in [
        ("obsT", [2, OBS_LEN * BSH], BF16),
        ("ys0", [2, 3 * BSH], F32),
        ("w0", [2, BSH], F32),
        ("b3b", [2, BSH], F32),
        ("lhsT_main", [99, 256], BF16),
        ("lhsT_scan", [67, 256], BF16),
        ("lhsT_wc", [64, 32], BF16),
        ("lhsT_w2", [32, 32], BF16),
        ("lhsT_w3", [96, 2], BF16),
        ("BCT", [32, TS], F32),
        ("b2c", [32, 1], F32),
        ("ident2", [2, 2], F32),
        ("zinit", [99, BSH], BF16),
    ]:
        din[name] = nc.dram_tensor(name, shape, dt, kind="ExternalInput")
    y_out = nc.dram_tensor("y", [BSH, 2 * TS], F32, kind="ExternalOutput")

    with _TileContextSW(nc) as tc:
        with (
            tc.tile_pool(name="consts", bufs=1) as cp,
            tc.tile_pool(name="work", bufs=2) as wk,
            tc.tile_pool(name="pg", bufs=2, space="PSUM") as pg,
            tc.tile_pool(name="pmix", bufs=2, space="PSUM") as pmix,
            tc.tile_pool(name="pyp", bufs=1, space="PSUM") as pyp,
            tc.tile_pool(name="pout", bufs=1, space="PSUM") as pout,
        ):
            # --- constants into SBUF
            sb = {}
            for name, t in din.items():
                sb[name] = cp.tile(list(t.shape), t.dtype, tag=name, name=name)
                nc.sync.dma_start(out=sb[name], in_=t[:, :])

            # Z: the gate-matmul rhs supertile.
            # rows [h 0:64 | a2 64:96 | w 96:98 | ones 98]  (all bf16);
            # the scan uses rows 64:66 for x with its own ones row at 66.
            Z = cp.tile([99, BSH], BF16, tag="Z")
            ch = cp.tile([64, BSH], F32, tag="ch")     # c/2
            ys = cp.tile([2, (TS + 3) * BSH], F32, tag="ys")
            O_sb = cp.tile([BSH, 2 * TS], F32, tag="O_sb")

            nc.sync.dma_start(out=Z[:, :], in_=din["zinit"][:, :])
            nc.vector.memset(ch, 0.0)
            nc.sync.dma_start(out=ys[:, 0 : 3 * BSH], in_=din["ys0"][:, :])

            O_ps = pout.tile([BSH, 2 * TS], F32, tag="O_ps")

            c15 = cp.tile([2, BSH], F32, tag="c15")
            cm05 = cp.tile([2, BSH], F32, tag="cm05")
            nc.vector.memset(c15, 1.5)
            nc.vector.memset(cm05, -0.5)

            lmain, lscan = sb["lhsT_main"], sb["lhsT_scan"]
            obsT = sb["obsT"]
            Zh = Z[0:64, :]      # h rows
            Zxs = Z[64:66, :]    # x rows (scan)
            Za2 = Z[64:96, :]    # a2 rows (main)
            Zw = Z[96:98, :]     # w rows (main)

            pend = {"wn": None}

            def state_update(G):
                """sigmoid(G) -> new ch (c/2) and h (Z rows 0:64, bf16)."""
                S = wk.tile([128, 256], F32, tag="S")
                nc.scalar.activation(out=S, in_=G, func=AF.Sigmoid)
                S_f, S_i = S[0:64, 0:128], S[64:128, 0:128]
                S_o, S_g = S[0:64, 128:256], S[64:128, 128:256]
                tt = wk.tile([64, BSH], F32, tag="tt")
                nc.vector.scalar_tensor_tensor(
                    out=tt, in0=S_g, scalar=0.5, in1=S_i,
                    op0=ALU.subtract, op1=ALU.mult)
                m2 = wk.tile([64, BSH], F32, tag="m2")
                nc.vector.tensor_mul(out=m2, in0=S_f, in1=ch)
                nc.vector.tensor_add(out=ch, in0=tt, in1=m2)
                th = wk.tile([64, BSH], F32, tag="th")
                th_i = nc.scalar.activation(out=th, in_=ch, func=AF.Tanh,
                                            scale=2.0)
                if pend["wn"] is not None:
                    # bf16 w for the next gate matmul: ScE Copy, slotted into
                    # the gap after Tanh so it never delays the ACT chain
                    cast_i = nc.scalar.copy(out=Zw, in_=pend["wn"])
                    add_dep_helper(cast_i.ins, th_i.ins, sync=False,
                                   reason="Zw cast after tanh on ScE")
                    pend["wn"] = None
                nc.vector.tensor_mul(out=Zh, in0=S_o, in1=th)

            # --- warmup scan over the observed trajectory (K=67 gate matmuls)
            G_cur = None
            for k in range(OBS_LEN):
                nc.vector.tensor_copy(Zxs, obsT[:, k * BSH : (k + 1) * BSH])
                G = pg.tile([128, 256], F32, tag="G")
                nc.tensor.matmul(G[:, 0:128], lscan[:, 0:128], Z[0:67, :],
                                 start=True, stop=True)
                nc.tensor.matmul(G[:, 128:256], lscan[:, 128:256],
                                 Z[0:67, :], start=True, stop=True)
                state_update(G)

            # --- 64 output steps (software-pipelined)
            # Gates for step t+1 are ONE matmul per half with rhs Z:
            #   G(t+1) = [WxW3 | Wh | Wx | b] . [a2_t; h_t; w_t; 1]
            # using y_t = W3.a2_t + w_t, so Wx.y_t = WxW3.a2_t + Wx.w_t and the
            # only on-path dependency is a2_t (w_t is known since step t-1).
            w_cur = cp.tile([2, BSH], F32, tag="w0_t")
            nc.sync.dma_start(out=w_cur, in_=din["w0"][:, :])
            nc.vector.tensor_copy(Zw, w_cur)  # bf16 w for the gate matmul

            for t in range(TS):
                if t > 0:
                    state_update(G_cur)
                build_next = t < TS - 1
                P1 = pmix.tile([32, BSH], F32, tag="P1")
                nc.tensor.matmul(P1, sb["lhsT_wc"], Zh,
                                 start=True, stop=True)
                a1 = wk.tile([32, BSH], BF16, tag="a1")
                nc.scalar.activation(out=a1, in_=P1, func=AF.Prelu,
                                     bias=sb["BCT"][:, t : t + 1], alpha=0.1)
                P2 = pmix.tile([32, BSH], F32, tag="P2")
                nc.tensor.matmul(P2, sb["lhsT_w2"], a1, start=True, stop=True)
                nc.scalar.activation(out=Za2, in_=P2, func=AF.Prelu,
                                     bias=sb["b2c"], alpha=0.1)
                if build_next:
                    G_next = pg.tile([128, 256], F32, tag="G")
                    nc.tensor.matmul(G_next[:, 0:128], lmain[:, 0:128], Z,
                                     start=True, stop=True)
                    nc.tensor.matmul(G_next[:, 128:256], lmain[:, 128:256], Z,
                                     start=True, stop=True)
                    G_cur = G_next
                PY = pyp.tile([2, BSH], F32, tag="PY")
                nc.tensor.matmul(PY, sb["lhsT_w3"][64:96, :], Za2,
                                 start=True, stop=True)
                ys_s = ys[:, (t + 3) * BSH : (t + 4) * BSH]
                yadd_i = nc.vector.tensor_add(out=ys_s, in0=PY, in1=w_cur)
                # accumulate transposed output column pair (off critical path)
                nc.tensor.matmul(O_ps[:, 2 * t : 2 * t + 2], ys_s,
                                 sb["ident2"], is_transpose=True,
                                 start=True, stop=True)
                # w for step t+1 = 1.5*y_t - 0.5*y_{t-2} + b3  (off critical path)
                u = wk.tile([2, BSH], F32, tag="u")
                nc.vector.scalar_tensor_tensor(
                    out=u, in0=ys_s, scalar=1.5, in1=sb["b3b"],
                    op0=ALU.mult, op1=ALU.add)
                wn = wk.tile([2, BSH], F32, tag="wn")
                nc.vector.scalar_tensor_tensor(
                    out=wn, in0=ys[:, (t + 1) * BSH : (t + 2) * BSH],
                    scalar=-0.5, in1=u, op0=ALU.mult, op1=ALU.add)
                if build_next:
                    pend["wn"] = wn
                w_cur = wn

            nc.scalar.copy(out=O_sb, in_=O_ps)
            nc.sync.dma_start(out=y_out[:, :], in_=O_sb)

    _split_multi_waits(nc)
    return nc


_CACHED = {}


def _get_nc():
    if "nc" not in _CACHED:
        _CACHED["nc"] = _build_bass()
    return _CACHED["nc"]


def _in_maps(inputs):
    f32 = np.float32
    W = {k: np.ascontiguousarray(np.asarray(v), dtype=f32) for k, v in inputs.items()
         if k != "teom"}
    tabs = _host_tables(W)
    b3 = tabs.pop("b3")
    obsv = W["obsv"]  # [1024, 8, 2]

    const_part = {k: np.ascontiguousarray(v) for k, v in tabs.items()}
    const_part["b3b"] = np.ascontiguousarray(
        np.broadcast_to(b3[:, None], (2, BSH)), dtype=f32)

    maps = []
    for i in range(N_CORES):
        sh = obsv[i * BSH : (i + 1) * BSH]  # [128, 8, 2]
        m = dict(const_part)
        import ml_dtypes
        m["obsT"] = np.ascontiguousarray(
            sh.transpose(2, 1, 0).reshape(2, OBS_LEN * BSH).astype(ml_dtypes.bfloat16))
        m["ys0"] = np.ascontiguousarray(
            sh[:, 5:8, :].transpose(2, 1, 0).reshape(2, 3 * BSH))
        w0 = 1.5 * sh[:, 7, :] - 0.5 * sh[:, 5, :] + b3  # [128, 2]
        m["w0"] = np.ascontiguousarray(w0.T)
        maps.append(m)
    return maps


def run(inputs, trace=False, **kw):
    nc = _get_nc()
    res = run_bass_kernel_spmd(
        nc, _in_maps(inputs), core_ids=list(range(N_CORES)), trace=trace, **kw)
    ys = [r["y"].reshape(BSH, TS, 2) for r in res.results]
    y = np.concatenate(ys, axis=0)
    c = np.zeros((BS, TS), np.float32)
    return (y, c), res


def kernel(**inputs):
    (y, c), _ = run(inputs)
    return y, c


# revision 59
# speedup vs baseline: 1.2254x; 1.0619x over previous
"""Trainium2 Bass kernel for nn_NavigationNet.

Math notes (validated host-side vs the jax reference, rel-l2 ~3e-6):

- `teom` is never used by the network (the conv path is dead code): the
  teom-LSTM consumes a hard-zero input with zero initial state, so its
  hidden-state trajectory is identical for every batch element.  Its entire
  contribution reduces to a precomputable per-step bias vector, which we fold
  through Wtf/Wmix/W1 into a [32, 64] table `BC` feeding the first MLP layer.
- `c_out` is exactly zeros.
- The mix head `W1 @ (Wmix[:,64:] @ (Wof @ h))` collapses into one [32,64]
  matrix `Wc`.
- All four LSTM gates use a single sigmoid activation per step: the g-gate
  pre-activation rows are pre-scaled by 2 so tanh(g) = 2*sigmoid(2g) - 1,
  and the cell state is tracked as c/2 so the trailing *2 folds into the
  Tanh activation's free `scale` operand.
- The y feedback `y_t = o_t + 1.5*y_{t-1} - 0.5*y_{t-3}` is kept in a
  [2, 67*128] ring (3 seed slots hold obsv[:, 5:8]).

Sharding: pure data parallel, batch 1024 -> 8 cores x 128.  Per core the
layout is feature-on-partition, batch-on-free (128 batch lanes in the free
dimension), so every op in the recurrence is a small [P<=128, 128] tile op.
"""

import numpy as np

import concourse.bass as bass
import concourse.mybir as mybir
import concourse.tile as tile
from concourse.bass_utils import run_bass_kernel_spmd
from concourse.tile import add_dep_helper
from concourse.vector_clock import ScopedClock

F32 = mybir.dt.float32
BF16 = mybir.dt.bfloat16
AF = mybir.ActivationFunctionType
ALU = mybir.AluOpType

BS, OBS_LEN, TS, HID = 1024, 8, 64, 64
N_CORES = 8
BSH = BS // N_CORES  # 128 batch per core


# ---------------------------------------------------------------------------
# walrus in this container rejects >1 sync-wait per instruction; split them.
class _TileContextSW(tile.TileContext):
    def _drain_and_barrier(self, tick_clock, wait_clock):
        probe = self.nc.sync.nop(hint="wait_probe", nofuse=True)
        wait_clock.add_sem_waits(
            probe.ins, ScopedClock({None: tick_clock.global_clock})
        )
        si = probe.ins.sync_info
        waits = list(si.on_wait) if si is not None and si.on_wait else []
        probe.ins.sync_info = None
        for w in waits:
            n = self.nc.sync.nop(hint="split_wait", nofuse=True)
            n.ins.sync_info = mybir.SyncInfo(on_wait=[w], on_update=[])
        self.nc.sync.drain()
        self.nc.all_engine_barrier()
        assert self.sems is not None
        popped = self.nc._tile_sem_poison_stack.pop()
        assert popped is self._sem_poison
        self.nc.clear_and_free_semaphores(list(self.sems.allocated().values()))
        self.nc.all_engine_barrier()


def _split_multi_waits(nc):
    for fn in nc.m.functions:
        for blk in fn.blocks:
            insts = blk.instructions
            out = []
            changed = False
            for inst in insts:
                si = inst.sync_info
                waits = list(si.on_wait) if si is not None and si.on_wait else []
                if len(waits) > 1:
                    for j, w in enumerate(waits[:-1]):
                        out.append(
                            mybir.InstNoOp(
                                name=f"{inst.name}-sw{j}",
                                engine=inst.engine,
                                ins=[],
                                outs=[],
                                sync_info=mybir.SyncInfo(on_wait=[w], on_update=[]),
                                bass_nofuse=True,
                            )
                        )
                    inst.sync_info = mybir.SyncInfo(
                        on_wait=[waits[-1]],
                        on_update=list(si.on_update) if si.on_update else [],
                    )
                    changed = True
                out.append(inst)
            if changed:
                insts[:] = out


# ---------------------------------------------------------------------------
# Host-side constant folding (float32 throughout, like the reference).
def _host_tables(W):
    f32 = np.float32
    sig = lambda v: (1.0 / (1.0 + np.exp(-v, dtype=np.float64))).astype(f32)
    tanh = lambda v: np.tanh(v.astype(np.float64)).astype(f32)

    b_o = W["bih_o"] + W["bhh_o"]
    b_t = W["bih_t"] + W["bhh_t"]
    Wmix_t = W["Wmix"][:, :64]
    Wmix_o = W["Wmix"][:, 64:]
    base = Wmix_o @ W["bof"] + W["bmix"]

    h = np.zeros(HID, f32)
    c = np.zeros(HID, f32)
    BC = np.zeros((32, TS), f32)
    for t in range(TS):
        g = W["Whh_t"] @ h + b_t
        i, f, gg, o = g[:64], g[64:128], g[128:192], g[192:256]
        c = sig(f) * c + sig(i) * tanh(gg)
        h = sig(o) * tanh(c)
        ts_ = W["Wtf"] @ h + W["btf"]
        BC[:, t] = W["W1"] @ (Wmix_t @ ts_ + base) + W["b1"]
    Wc = W["W1"] @ Wmix_o @ W["Wof"]

    Wh = W["Whh_o"].copy()  # [256, 64]
    Wx = W["Wih_o"].copy()  # [256, 2]
    bb = b_o.copy()
    Wh[128:192] *= 2.0  # g-gate rows via tanh(x) = 2 sig(2x) - 1
    Wx[128:192] *= 2.0
    bb[128:192] *= 2.0
    # gate-row order [f; i; o; g2]: keeps every later tensor-tensor pair on
    # matching base partitions (f/o at 0, i/g at 64).
    perm = np.concatenate([
        np.arange(64, 128), np.arange(0, 64),
        np.arange(192, 256), np.arange(128, 192)])
    Wh, Wx, bb = Wh[perm], Wx[perm], bb[perm]
    WxW3 = Wx @ W["W3"]  # [256, 32]
    # rhs supertile Z rows: [h 0:64 | a2 64:96 | w 96:98 | ones 98]
    # (the scan reuses rows 64:66 for x with its own ones row at 66)
    Wmain = np.concatenate([Wh, WxW3, Wx, bb[:, None]], axis=1)  # [256, 99]
    Wscan = np.concatenate([Wh, Wx, bb[:, None]], axis=1)        # [256, 67]

    import ml_dtypes
    bf16 = ml_dtypes.bfloat16
    zinit = np.zeros((99, BSH), np.float32)
    zinit[66, :] = 1.0
    zinit[98, :] = 1.0
    # lhsT_w3 padded so its slice starts at base partition 64 to match the
    # a2 rows of Z (matmul requires equal base partitions).
    w3_p = np.zeros((96, 2), np.float32)
    w3_p[64:96] = W["W3"].T
    return dict(
        lhsT_main=np.ascontiguousarray(Wmain.T.astype(bf16)),  # [99, 256]
        lhsT_scan=np.ascontiguousarray(Wscan.T.astype(bf16)),  # [67, 256]
        lhsT_wc=np.ascontiguousarray(Wc.T.astype(bf16)),       # [64, 32]
        lhsT_w2=np.ascontiguousarray(W["W2"].T.astype(bf16)),  # [32, 32]
        lhsT_w3=np.ascontiguousarray(w3_p.astype(bf16)),       # [96, 2]
        BCT=BC,                                     # [32, 64]
        b2c=np.ascontiguousarray(W["b2"][:, None]),  # [32, 1]
        b3=W["b3"],                                  # [2]
        ident2=np.eye(2, dtype=f32),                 # [2, 2]
        zinit=np.ascontiguousarray(zinit.astype(bf16)),  # [67, BSH]
    )


# ---------------------------------------------------------------------------
def _build_bass():
    nc = bass.Bass()

    din = {}
    for name, shape, dt in [
        ("obsT", [2, OBS_LEN * BSH], BF16),
        ("ys0", [2, 3 * BSH], F32),
        ("w0", [2, BSH], F32),
        ("b3b", [2, BSH], F32),
        ("lhsT_main", [99, 256], BF16),
        ("lhsT_scan", [67, 256], BF16),
        ("lhsT_wc", [64, 32], BF16),
        ("lhsT_w2", [32, 32], BF16),
        ("lhsT_w3", [96, 2], BF16),
        ("BCT", [32, TS], F32),
        ("b2c", [32, 1], F32),
        ("ident2", [2, 2], F32),
        ("zinit", [99, BSH], BF16),
    ]:
        din[name] = nc.dram_tensor(name, shape, dt, kind="ExternalInput")
    y_out = nc.dram_tensor("y", [BSH, 2 * TS], F32, kind="ExternalOutput")

    with _TileContextSW(nc) as tc:
        with (
            tc.tile_pool(name="consts", bufs=1) as cp,
            tc.tile_pool(name="work", bufs=2) as wk,
            tc.tile_pool(name="pg", bufs=2, space="PSUM") as pg,
            tc.tile_pool(name="pmix", bufs=2, space="PSUM") as pmix,
            tc.tile_pool(name="pyp", bufs=1, space="PSUM") as pyp,
            tc.tile_pool(name="pout", bufs=1, space="PSUM") as pout,
        ):
            # --- constants into SBUF
            sb = {}
            for name, t in din.items():
                sb[name] = cp.tile(list(t.shape), t.dtype, tag=name, name=name)
                nc.sync.dma_start(out=sb[name], in_=t[:, :])

            # Z: the gate-matmul rhs supertile.
            # rows [h 0:64 | a2 64:96 | w 96:98 | ones 98]  (all bf16);
            # the scan uses rows 64:66 for x with its own ones row at 66.
            Z = cp.tile([99, BSH], BF16, tag="Z")
            ch = cp.tile([64, BSH], F32, tag="ch")     # c/2
            ys = cp.tile([2, (TS + 3) * BSH], F32, tag="ys")
            O_sb = cp.tile([BSH, 2 * TS], F32, tag="O_sb")

            nc.sync.dma_start(out=Z[:, :], in_=din["zinit"][:, :])
            nc.vector.memset(ch, 0.0)
            nc.sync.dma_start(out=ys[:, 0 : 3 * BSH], in_=din["ys0"][:, :])

            O_ps = pout.tile([BSH, 2 * TS], F32, tag="O_ps")

            c15 = cp.tile([2, BSH], F32, tag="c15")
            cm05 = cp.tile([2, BSH], F32, tag="cm05")
            nc.vector.memset(c15, 1.5)
            nc.vector.memset(cm05, -0.5)

            lmain, lscan = sb["lhsT_main"], sb["lhsT_scan"]
            obsT = sb["obsT"]
            Zh = Z[0:64, :]      # h rows
            Zxs = Z[64:66, :]    # x rows (scan)
            Za2 = Z[64:96, :]    # a2 rows (main)
            Zw = Z[96:98, :]     # w rows (main)

            pend = {"uw": None}

            def state_update(G):
                """sigmoid(G) -> new ch (c/2) and h (Z rows 0:64, bf16)."""
                S = wk.tile([128, 256], F32, tag="S")
                nc.scalar.activation(out=S, in_=G, func=AF.Sigmoid)
                S_f, S_i = S[0:64, 0:128], S[64:128, 0:128]
                S_o, S_g = S[0:64, 128:256], S[64:128, 128:256]
                tt = wk.tile([64, BSH], F32, tag="tt")
                nc.vector.scalar_tensor_tensor(
                    out=tt, in0=S_g, scalar=0.5, in1=S_i,
                    op0=ALU.subtract, op1=ALU.mult)
                m2 = wk.tile([64, BSH], F32, tag="m2")
                nc.vector.tensor_mul(out=m2, in0=S_f, in1=ch)
                nc.vector.tensor_add(out=ch, in0=tt, in1=m2)
                th = wk.tile([64, BSH], F32, tag="th")
                th_i = nc.scalar.activation(out=th, in_=ch, func=AF.Tanh,
                                            scale=2.0)
                hm_i = nc.vector.tensor_mul(out=Zh, in0=S_o, in1=th)
                if pend["uw"] is not None:
                    # deferred w-recurrence of the PREVIOUS step: VE ops queued
                    # behind this state update so they fill the mix-phase gap,
                    # plus the bf16 cast on ScE in the post-tanh bubble.
                    ys_prev, slot_prev, want_cast = pend["uw"]
                    pend["uw"] = None
                    u = wk.tile([2, BSH], F32, tag="u")
                    u_i = nc.vector.scalar_tensor_tensor(
                        out=u, in0=ys_prev, scalar=1.5, in1=sb["b3b"],
                        op0=ALU.mult, op1=ALU.add)
                    add_dep_helper(u_i.ins, hm_i.ins, sync=False,
                                   reason="w-recurrence after state update")
                    wn = wk.tile([2, BSH], F32, tag="wn")
                    nc.vector.scalar_tensor_tensor(
                        out=wn, in0=slot_prev,
                        scalar=-0.5, in1=u, op0=ALU.mult, op1=ALU.add)
                    if want_cast:
                        cast_i = nc.scalar.copy(out=Zw, in_=wn)
                        add_dep_helper(cast_i.ins, th_i.ins, sync=False,
                                       reason="Zw cast after tanh on ScE")
                    state_update.w_out = wn
                else:
                    state_update.w_out = None

            # --- warmup scan over the observed trajectory (K=67 gate matmuls)
            G_cur = None
            nc.vector.tensor_copy(Zxs, obsT[:, 0:BSH])
            for k in range(OBS_LEN):
                G = pg.tile([128, 256], F32, tag="G")
                nc.tensor.matmul(G[:, 0:128], lscan[:, 0:128], Z[0:67, :],
                                 start=True, stop=True)
                nc.tensor.matmul(G[:, 128:256], lscan[:, 128:256],
                                 Z[0:67, :], start=True, stop=True)
                if k + 1 < OBS_LEN:
                    # next step's x can load while this state update runs
                    nc.vector.tensor_copy(
                        Zxs, obsT[:, (k + 1) * BSH : (k + 2) * BSH])
                state_update(G)

            # --- 64 output steps (software-pipelined)
            # Gates for step t+1 are ONE matmul per half with rhs Z:
            #   G(t+1) = [WxW3 | Wh | Wx | b] . [a2_t; h_t; w_t; 1]
            # using y_t = W3.a2_t + w_t, so Wx.y_t = WxW3.a2_t + Wx.w_t and the
            # only on-path dependency is a2_t (w_t is known since step t-1).
            w_cur = cp.tile([2, BSH], F32, tag="w0_t")
            nc.sync.dma_start(out=w_cur, in_=din["w0"][:, :])
            nc.vector.tensor_copy(Zw, w_cur)  # bf16 w for the gate matmul

            for t in range(TS):
                if t > 0:
                    state_update(G_cur)
                    if state_update.w_out is not None:
                        w_cur = state_update.w_out
                build_next = t < TS - 1
                P1 = pmix.tile([32, BSH], F32, tag="P1")
                nc.tensor.matmul(P1, sb["lhsT_wc"], Zh,
                                 start=True, stop=True)
                a1 = wk.tile([32, BSH], BF16, tag="a1")
                nc.scalar.activation(out=a1, in_=P1, func=AF.Prelu,
                                     bias=sb["BCT"][:, t : t + 1], alpha=0.1)
                P2 = pmix.tile([32, BSH], F32, tag="P2")
                nc.tensor.matmul(P2, sb["lhsT_w2"], a1, start=True, stop=True)
                nc.scalar.activation(out=Za2, in_=P2, func=AF.Prelu,
                                     bias=sb["b2c"], alpha=0.1)
                if build_next:
                    G_next = pg.tile([128, 256], F32, tag="G")
                    nc.tensor.matmul(G_next[:, 0:128], lmain[:, 0:128], Z,
                                     start=True, stop=True)
                    nc.tensor.matmul(G_next[:, 128:256], lmain[:, 128:256], Z,
                                     start=True, stop=True)
                    G_cur = G_next
                PY = pyp.tile([2, BSH], F32, tag="PY")
                nc.tensor.matmul(PY, sb["lhsT_w3"][64:96, :], Za2,
                                 start=True, stop=True)
                ys_s = ys[:, (t + 3) * BSH : (t + 4) * BSH]
                yadd_i = nc.vector.tensor_add(out=ys_s, in0=PY, in1=w_cur)
                # accumulate transposed output column pair (off critical path)
                nc.tensor.matmul(O_ps[:, 2 * t : 2 * t + 2], ys_s,
                                 sb["ident2"], is_transpose=True,
                                 start=True, stop=True)
                # w for step t+1 = 1.5*y_t - 0.5*y_{t-2} + b3: deferred into
                # the next state_update so it stays off the critical path.
                if build_next:
                    pend["uw"] = (ys_s, ys[:, (t + 1) * BSH : (t + 2) * BSH],
                                  t < TS - 2)

            nc.scalar.copy(out=O_sb, in_=O_ps)
            nc.sync.dma_start(out=y_out[:, :], in_=O_sb)

    _split_multi_waits(nc)
    return nc


_CACHED = {}


def _get_nc():
    if "nc" not in _CACHED:
        _CACHED["nc"] = _build_bass()
    return _CACHED["nc"]


def _in_maps(inputs):
    f32 = np.float32
    W = {k: np.ascontiguousarray(np.asarray(v), dtype=f32) for k, v in inputs.items()
         if k != "teom"}
    tabs = _host_tables(W)
    b3 = tabs.pop("b3")
    obsv = W["obsv"]  # [1024, 8, 2]

    const_part = {k: np.ascontiguousarray(v) for k, v in tabs.items()}
    const_part["b3b"] = np.ascontiguousarray(
        np.broadcast_to(b3[:, None], (2, BSH)), dtype=f32)

    maps = []
    for i in range(N_CORES):
        sh = obsv[i * BSH : (i + 1) * BSH]  # [128, 8, 2]
        m = dict(const_part)
        import ml_dtypes
        m["obsT"] = np.ascontiguousarray(
            sh.transpose(2, 1, 0).reshape(2, OBS_LEN * BSH).astype(ml_dtypes.bfloat16))
        m["ys0"] = np.ascontiguousarray(
            sh[:, 5:8, :].transpose(2, 1, 0).reshape(2, 3 * BSH))
        w0 = 1.5 * sh[:, 7, :] - 0.5 * sh[:, 5, :] + b3  # [128, 2]
        m["w0"] = np.ascontiguousarray(w0.T)
        maps.append(m)
    return maps


def run(inputs, trace=False, **kw):
    nc = _get_nc()
    res = run_bass_kernel_spmd(
        nc, _in_maps(inputs), core_ids=list(range(N_CORES)), trace=trace, **kw)
    ys = [r["y"].reshape(BSH, TS, 2) for r in res.results]
    y = np.concatenate(ys, axis=0)
    c = np.zeros((BS, TS), np.float32)
    return (y, c), res


def kernel(**inputs):
    (y, c), _ = run(inputs)
    return y, c
